# revision 14
# baseline (speedup 1.0000x reference)
"""v3: k-major SBUF layouts with per-DMA-contiguous DRAM packing.

Each operand lives in ONE SBUF tile with layout [128, k_chunks * free]:
  xt_sb [P, kd*cap]   xt_sb[p, s*cap+t] = x[tok_t, s*128+p]
  w1_sb [P, kd*FF]    w1_sb[p, s*FF+f]  = w1[e][f, s*128+p]
  w2_sb [P, mf*D]     w2_sb[p, s*D+d]   = w2[e][d, s*128+p]
  h_sb  [P, mf*cap]   written by stage-1 silu, consumed by stage-2
DRAM inputs are packed so every DMA chunk is a fully contiguous region
(chunk index outermost). Output y is written as fp16 to halve traffic.
"""

import numpy as np
import ml_dtypes

import concourse.bass as bass
import concourse.mybir as mybir
from concourse.bacc import Bacc
from concourse.tile import TileContext
from concourse.bass_utils import run_bass_kernel_spmd

B, S, D = 2, 1024, 1024
E, TOPK, FF = 8, 2, 2048
N_CORES = 8
P = 128
KD = D // P      # 8
MF = FF // P     # 16
ND = D // 512    # 2
DEFAULT_CAP = 640
KSUB = 1         # k-chunks per matmul instruction (>1 is fp8-DoubleRow-only)
GW = 4           # stage-1 m-group width
XCH = 4          # xt DMA chunks (along k)
W1CH = 8         # w1 DMA chunks (along FF)
W2CH = 4         # w2 DMA chunks (along D)

TRACE = False
LAST_RESULTS = None

_nc_cache = {}


def _n1_sizes(cap):
    sizes = []
    rem = cap
    while rem > 0:
        sz = min(512, rem)
        sizes.append(sz)
        rem -= sz
    return sizes


def _emit_rep(nc, pools, aps, cap, r, ksub):
    wpool, xpool, hpool, ypool, ph, py = pools
    xt, w1t, w2t, cw, y = aps
    nt = cap // P
    ngrp = MF // GW

    bf16 = mybir.dt.bfloat16
    f32 = mybir.dt.float32
    f16 = mybir.dt.float16

    # ---- single-tile operands; every DMA reads a contiguous DRAM chunk ----
    xts = xpool.tile([P, KD * cap], bf16, name=f"xts_{r}", tag="xts")
    x3 = xts[:].rearrange("p (s t) -> p s t", s=KD)
    w1s = wpool.tile([P, KD * FF], bf16, name=f"w1s_{r}", tag="w1s")
    w13 = w1s[:].rearrange("p (s f) -> p s f", s=KD)
    w1cw = FF // W1CH
    # interleave: x k-pair then the w1 chunks the next m-group needs
    for c in range(XCH):
        ks = KD // XCH
        nc.sync.dma_start(
            out=x3[:, c * ks:(c + 1) * ks, :],
            in_=xt[c, :, :].rearrange("p (s t) -> p s t", s=ks))
        for wc in range(c * W1CH // XCH, (c + 1) * W1CH // XCH):
            nc.sync.dma_start(
                out=w13[:, :, wc * w1cw:(wc + 1) * w1cw],
                in_=w1t[wc, :, :].rearrange("p (s f) -> p s f", s=KD))

    cwt = xpool.tile([P, nt], f32, name=f"cwt_{r}", tag="cwt")
    nc.sync.dma_start(out=cwt[:], in_=cw[:, :])

    w2s = wpool.tile([P, MF * D], bf16, name=f"w2s_{r}", tag="w2s")
    w23 = w2s[:].rearrange("p (s d) -> p s d", s=MF)
    w2cw = D // W2CH
    for c in range(W2CH):
        nc.sync.dma_start(
            out=w23[:, :, c * w2cw:(c + 1) * w2cw],
            in_=w2t[c, :, :].rearrange("p (s d) -> p s d", s=MF))

    h = hpool.tile([P, MF * cap], bf16, name=f"h_{r}", tag="h")
    h3 = h[:].rearrange("p (s t) -> p s t", s=MF)

    # ---- stage 1 ----
    for ni, nsz in enumerate(_n1_sizes(cap)):
        n0 = ni * 512
        for g in range(ngrp):
            pts = [
                ph.tile([P, 512], f32, name=f"ps1_{g}_{i}_{ni}_{r}", tag=f"ps1_{i}")
                for i in range(GW)
            ]
            for ks in range(0, KD, ksub):
                for i in range(GW):
                    m = g * GW + i
                    nc.tensor.matmul(
                        pts[i][:, 0:nsz],
                        lhsT=w13[:, ks:ks + ksub, m * P:(m + 1) * P],
                        rhs=x3[:, ks:ks + ksub, n0:n0 + nsz],
                        start=(ks == 0),
                        stop=(ks + ksub == KD),
                    )
            for i in range(GW):
                m = g * GW + i
                nc.scalar.activation(
                    h3[:, m, n0:n0 + nsz], pts[i][:, 0:nsz],
                    mybir.ActivationFunctionType.Silu,
                )

    # ---- stage 2 ----
    for t in range(nt):
        for d in range(ND):
            pt = py.tile([P, 512], f32, name=f"ps2_{t}_{d}_{r}", tag="ps2")
            for ss in range(0, MF, ksub):
                nc.tensor.matmul(
                    pt[:],
                    lhsT=h3[:, ss:ss + ksub, t * P:(t + 1) * P],
                    rhs=w23[:, ss:ss + ksub, d * 512:(d + 1) * 512],
                    start=(ss == 0),
                    stop=(ss + ksub == MF),
                )
            ysb = ypool.tile([P, 512], f16, name=f"y_{t}_{d}_{r}", tag="ysb")
            nc.scalar.activation(
                ysb[:], pt[:], mybir.ActivationFunctionType.Copy,
                scale=cwt[:, t:t + 1],
            )
            nc.sync.dma_start(
                out=y[d, t * P:(t + 1) * P, :], in_=ysb[:]
            )


def _make_graph(cap, reps=1, loop_iters=0, ksub=KSUB):
    assert cap % P == 0
    nt = cap // P
    bf16 = mybir.dt.bfloat16
    f32 = mybir.dt.float32

    nc = Bacc()
    f16 = mybir.dt.float16
    xt = nc.dram_tensor("xt", [XCH, P, (KD // XCH) * cap], bf16, kind="ExternalInput")
    w1t = nc.dram_tensor("w1t", [W1CH, P, KD * (FF // W1CH)], bf16, kind="ExternalInput")
    w2t = nc.dram_tensor("w2t", [W2CH, P, MF * (D // W2CH)], bf16, kind="ExternalInput")
    cw = nc.dram_tensor("cw", [P, nt], f32, kind="ExternalInput")
    y = nc.dram_tensor("y", [ND, cap, 512], f16, kind="ExternalOutput")
    aps = (xt, w1t, w2t, cw, y)

    with TileContext(nc) as tc:
        with (
            tc.tile_pool(name="wpool", bufs=1) as wpool,
            tc.tile_pool(name="xpool", bufs=1) as xpool,
            tc.tile_pool(name="hpool", bufs=1) as hpool,
            tc.tile_pool(name="ypool", bufs=4) as ypool,
            tc.tile_pool(name="ph", bufs=1, space="PSUM") as ph,
            tc.tile_pool(name="py", bufs=4, space="PSUM") as py,
        ):
            pools = (wpool, xpool, hpool, ypool, ph, py)
            if loop_iters > 0:
                with tc.For_i(0, loop_iters, 1):
                    for r in range(reps):
                        _emit_rep(nc, pools, aps, cap, r, ksub)
            else:
                for r in range(reps):
                    _emit_rep(nc, pools, aps, cap, r, ksub)
    return nc


def _get_nc(cap, reps=1, loop_iters=0, ksub=KSUB):
    key = (cap, reps, loop_iters, ksub)
    if key not in _nc_cache:
        nc = _make_graph(cap, reps, loop_iters, ksub)
        nc.finalize()
        _nc_cache[key] = nc
    return _nc_cache[key]


def _route(x, router_w):
    logits = x @ router_w.T
    m = logits.max(axis=-1, keepdims=True)
    ex = np.exp(logits - m, dtype=np.float32)
    probs = (ex / ex.sum(axis=-1, keepdims=True)).astype(np.float32)
    order = np.argsort(-probs, axis=-1, kind="stable")
    topk_idx = order[:, :TOPK].astype(np.int32)
    tw = np.take_along_axis(probs, topk_idx, axis=-1).astype(np.float32)
    topk_w = (tw / tw.sum(axis=-1, keepdims=True)).astype(np.float32)
    return probs, topk_idx, topk_w


def _kmajor_chunks(a2d, n_chunks):
    """[R, C] with R = kch*P -> [n_chunks, P, kch*(C/n_chunks)]: column-chunked
    k-major pack; each [i] slice is one contiguous DMA source."""
    R, C = a2d.shape
    kch = R // P
    ccw = C // n_chunks
    a = a2d.reshape(kch, P, n_chunks, ccw)        # [s, p, c, f]
    a = a.transpose(2, 1, 0, 3)                   # [c, p, s, f]
    return np.ascontiguousarray(a.reshape(n_chunks, P, kch * ccw))


def _xmajor_chunks(a2d, n_chunks):
    """[R, C] with R = kch*P -> [n_chunks, P, (kch/n_chunks)*C]: k-chunked
    pack for xt; each [i] slice is one contiguous DMA source."""
    R, C = a2d.shape
    kch = R // P
    ks = kch // n_chunks
    a = a2d.reshape(n_chunks, ks, P, C)           # [c, s, p, t]
    a = a.transpose(0, 2, 1, 3)                   # [c, p, s, t]
    return np.ascontiguousarray(a.reshape(n_chunks, P, ks * C))


def _prepare_in_maps(xf, w1, w2, topk_idx, topk_w):
    idx_lists, w_lists = [], []
    max_cnt = 0
    for e in range(E):
        hit = (topk_idx == e)
        sel = np.where(hit.any(axis=-1))[0]
        wsel = np.where(hit[sel, 0], topk_w[sel, 0], topk_w[sel, 1])
        idx_lists.append(sel)
        w_lists.append(wsel.astype(np.float32))
        max_cnt = max(max_cnt, len(sel))

    cap = max(DEFAULT_CAP, ((max_cnt + P - 1) // P) * P)
    nt = cap // P
    bf16 = ml_dtypes.bfloat16
    in_maps = []
    for e in range(E):
        sel, wsel = idx_lists[e], w_lists[e]
        cnt = len(sel)
        pad_sel = np.zeros(cap, dtype=np.int64)
        pad_sel[:cnt] = sel
        cwv = np.zeros(cap, dtype=np.float32)
        cwv[:cnt] = wsel
        xte = _xmajor_chunks(xf[pad_sel].T.astype(bf16), XCH)
        w1te = _kmajor_chunks(w1[e].T.astype(bf16), W1CH)
        w2te = _kmajor_chunks(w2[e].T.astype(bf16), W2CH)
        cwm = np.ascontiguousarray(cwv.reshape(nt, P).T)   # [P, nt]
        in_maps.append({"xt": xte, "w1t": w1te, "w2t": w2te, "cw": cwm})
    return in_maps, idx_lists, cap


def kernel(x, router_w, w1, w2):
    global LAST_RESULTS
    x = np.asarray(x, dtype=np.float32)
    router_w = np.asarray(router_w, dtype=np.float32)
    w1 = np.asarray(w1, dtype=np.float32)
    w2 = np.asarray(w2, dtype=np.float32)

    T = B * S
    xf = np.ascontiguousarray(x.reshape(T, D))
    probs, topk_idx, topk_w = _route(xf, router_w)
    in_maps, idx_lists, cap = _prepare_in_maps(xf, w1, w2, topk_idx, topk_w)

    nc = _get_nc(cap)
    res = run_bass_kernel_spmd(nc, in_maps, list(range(N_CORES)), trace=TRACE)
    LAST_RESULTS = res

    out = np.zeros((T, D), dtype=np.float32)
    for e in range(E):
        sel = idx_lists[e]
        cnt = len(sel)
        yr = np.asarray(res.results[e]["y"])               # [ND, cap, 512] f16
        ye = np.concatenate([yr[d] for d in range(ND)], axis=1).astype(np.float32)
        out[sel] += ye[:cnt]

    return (
        out.reshape(B, S, D),
        probs.reshape(B, S, E),
        topk_idx.reshape(B, S, TOPK),
        topk_w.reshape(B, S, TOPK),
    )
